# revision 1
# baseline (speedup 1.0000x reference)
"""Trainium2 Bass kernel for nn_CrossAttention_82429012345074.

8-head self-attention, B=2, N=4096, d_model=512, 8 heads x 64 dim.

Sharding: one head per NeuronCore (8 heads / 8 cores) — tensor parallel:
to_q/k/v column-parallel (each core gets its head's 64 rows of Wq/Wk/Wv),
to_out row-parallel (each core gets its head's 64 columns of Wo and emits a
partial [tok, 512] output). The unshard step sums the 8 partials + bias on
host.

Per-core device kernel (all matmuls in bf16, fp32 accumulation):
  xT = dma_transpose(x)                       # [512f, 8192t] in 4 chunks
  qT/kT = W @ xT                              # [64, 8192]
  v     = xT.T @ Wv.T (natural layout)        # [8192, 64] + ones column
  per (batch, 512-query group):
     sT[j,q] = kT_blk.T @ qT_grp              # PSUM, 3 key-blocks per chunk
     pT = exp(sT * scale)                     # ScalarE, PSUM->SBUF bf16
     o[65, q] += [v|1].T @ pT                 # accumulate over j; row 64 = denom
     oN = o[0:64] * (1/o[64]) broadcast       # normalized attention out (bf16)
     out[t, :] = oN_blk.T @ WoT_head          # partial output, DMA to DRAM
"""

import sys

sys.path.insert(0, "/opt/trn_rl_repo")

import numpy as np
import ml_dtypes

B, N, D, H, DH = 2, 4096, 512, 8, 64
TOK = B * N            # 8192
NQ = 512               # query-group width
NCH = D // 128         # 4 feature chunks of x
NJB = N // 128         # 32 key blocks per batch
NTB = TOK // 128       # 64 token blocks
JGS = [3] * 10 + [2]   # key-blocks per exp() chunk (sum = 32)
SCALE = DH ** -0.5
N_FILL = 2             # PE filler LDWEIGHTS per chunk (HAM warm-pinning)


def build_bass():
    from contextlib import ExitStack

    import concourse.bass as bass
    import concourse.mybir as mybir
    import concourse.tile as tile
    from concourse import bacc

    f32 = mybir.dt.float32
    bf16 = mybir.dt.bfloat16
    EXP = mybir.ActivationFunctionType.Exp

    nc = bacc.Bacc("TRN2", target_bir_lowering=False, num_devices=8)
    x_d = nc.dram_tensor("x", [NCH, 128, TOK], bf16, kind="ExternalInput")
    wq_d = nc.dram_tensor("wq", [D, DH], bf16, kind="ExternalInput")
    wk_d = nc.dram_tensor("wk", [D, DH], bf16, kind="ExternalInput")
    wv_d = nc.dram_tensor("wv", [D, DH], bf16, kind="ExternalInput")
    wo_d = nc.dram_tensor("wo", [DH, D], bf16, kind="ExternalInput")
    out_d = nc.dram_tensor("out", [TOK, D], f32, kind="ExternalOutput")

    with tile.TileContext(nc) as tc, ExitStack() as ctx:
        const = ctx.enter_context(tc.tile_pool(name="const", bufs=1))
        sb_p = ctx.enter_context(tc.tile_pool(name="sb_p", bufs=3))
        sb_io = ctx.enter_context(tc.tile_pool(name="sb_io", bufs=3))
        ps_s = ctx.enter_context(tc.tile_pool(name="ps_s", bufs=2, space="PSUM"))
        ps_sm = ctx.enter_context(tc.tile_pool(name="ps_sm", bufs=2, space="PSUM"))

        # Long-lived SBUF tensors
        xT = const.tile([128, NCH, TOK], bf16, name="xT")      # x transposed, 4 chunks
        qT = const.tile([64, TOK], bf16, name="qT")
        kT = const.tile([64, TOK], bf16, name="kT")
        vP = const.tile([128, NTB, DH + 1], bf16, name="vP")   # v blocks + ones col
        oN = const.tile([64, TOK], bf16, name="oN")            # normalized attn out^T
        wq = const.tile([128, NCH, DH], bf16, name="wq")
        wk = const.tile([128, NCH, DH], bf16, name="wk")
        wv = const.tile([128, NCH, DH], bf16, name="wv")
        wo = const.tile([64, D], bf16, name="wo")

        nc.vector.memset(vP[:, :, DH : DH + 1], 1.0)
        ones1 = const.tile([1, 1], f32, name="ones1")
        nc.vector.memset(ones1, 1.0)

        nc.sync.dma_start(out=wq, in_=wq_d[:].rearrange("(c p) d -> p c d", p=128))
        nc.sync.dma_start(out=wk, in_=wk_d[:].rearrange("(c p) d -> p c d", p=128))
        nc.sync.dma_start(out=wv, in_=wv_d[:].rearrange("(c p) d -> p c d", p=128))
        nc.sync.dma_start(out=wo, in_=wo_d[:])

        # Phase 0: load host-pre-transposed x chunks (contiguous, full-BW DMA)
        for c in range(NCH):
            for tt in range(TOK // 2048):
                t0 = tt * 2048
                nc.sync.dma_start(
                    out=xT[:, c, t0 : t0 + 2048],
                    in_=x_d[c, :, t0 : t0 + 2048],
                )

        # Phase 1: projections
        for g in range(TOK // NQ):
            t0 = g * NQ
            qp = ps_s.tile([64, NQ], f32, tag="s", name="qp")
            for c in range(NCH):
                nc.tensor.matmul(
                    qp, lhsT=wq[:, c, :], rhs=xT[:, c, t0 : t0 + NQ],
                    start=(c == 0), stop=(c == NCH - 1),
                )
            nc.vector.tensor_copy(out=qT[:, t0 : t0 + NQ], in_=qp)

            kp = ps_s.tile([64, NQ], f32, tag="s", name="kp")
            for c in range(NCH):
                nc.tensor.matmul(
                    kp, lhsT=wk[:, c, :], rhs=xT[:, c, t0 : t0 + NQ],
                    start=(c == 0), stop=(c == NCH - 1),
                )
            nc.vector.tensor_copy(out=kT[:, t0 : t0 + NQ], in_=kp)

            for t in range(NQ // 128):
                tb = g * (NQ // 128) + t
                vp = ps_sm.tile([128, DH], f32, tag="o", name="vp")
                for c in range(NCH):
                    nc.tensor.matmul(
                        vp, lhsT=xT[:, c, tb * 128 : tb * 128 + 128], rhs=wv[:, c, :],
                        start=(c == 0), stop=(c == NCH - 1),
                    )
                nc.vector.tensor_copy(out=vP[:, tb, 0:DH], in_=vp)

        # Phase 2+3: attention + output projection.
        #
        # oN holds the UNNORMALIZED attention output (bf16); the softmax
        # denominator (o row DH) is transposed into token-partition layout
        # via tiny K=1 matmuls, reciprocal'd wide, and applied as a
        # per-partition scalar fused into the projection's PSUM->SBUF copy.
        # Each group's epilogue (denominator transpose + projection) is
        # interleaved into the NEXT group's score loop so the in-order PE
        # queue never stalls on the normalize chain.
        def emit_denT(pq0, pden):
            denT = ps_sm.tile([128, 4], f32, tag="o", name="denT")
            for t in range(NQ // 128):
                nc.tensor.matmul(
                    denT[:, t : t + 1],
                    lhsT=pden[0:1, t * 128 : (t + 1) * 128],
                    rhs=ones1, start=True, stop=True,
                )
            recT = sb_io.tile([128, 4], f32, name="recT")
            nc.vector.reciprocal(recT, denT)
            return recT

        def emit_fp_one(pq0, t, recT):
            tt0 = pq0 + t * 128
            fp = ps_sm.tile([128, D], f32, tag="o", name="fp")
            nc.tensor.matmul(
                fp, lhsT=oN[:, tt0 : tt0 + 128], rhs=wo, start=True, stop=True
            )
            ob = sb_io.tile([128, D], f32, name="ob")
            nc.vector.tensor_scalar_mul(ob, in0=fp, scalar1=recT[:, t : t + 1])
            nc.sync.dma_start(out=out_d[tt0 : tt0 + 128, :], in_=ob)

        pending = None  # (q0, den tile) of the previous group
        for b in range(B):
            for qg in range(N // NQ):
                q0 = b * N + qg * NQ
                o = ps_sm.tile([DH + 1, NQ], f32, tag="o", name="o")
                jb = 0
                recT = None
                # PV is deferred one chunk: the PE queue reads
                # [QK(g), PV(g-1), QK(g+1), PV(g), ...] so exp(g) on ScalarE
                # overlaps PV(g-1)/QK(g+1) instead of serializing the chunk.
                pv_queue = None  # (p tile, jb, gsz) awaiting emission
                def flush_pv(pv):
                    p, pjb, pgsz = pv
                    for i in range(pgsz):
                        jbg = b * NJB + pjb + i
                        nc.tensor.matmul(
                            o, lhsT=vP[:, jbg, :], rhs=p[:, i, :],
                            start=(pjb + i == 0), stop=(pjb + i == NJB - 1),
                        )
                for gi, gsz in enumerate(JGS):
                    s = ps_s.tile([128, 3, NQ], f32, tag="s", name="s")
                    for i in range(gsz):
                        j0 = b * N + (jb + i) * 128
                        nc.tensor.matmul(
                            s[:, i, :], lhsT=kT[:, j0 : j0 + 128],
                            rhs=qT[:, q0 : q0 + NQ], start=True, stop=True,
                        )
                    p = sb_p.tile([128, 3, NQ], bf16, name="p")
                    nc.scalar.activation(
                        out=p[:, 0:gsz, :], in_=s[:, 0:gsz, :], func=EXP, scale=SCALE
                    )
                    if pv_queue is not None:
                        flush_pv(pv_queue)
                    pv_queue = (p, jb, gsz)
                    jb += gsz
                    # PE filler: keeps TensorE strictly busier than ScalarE so
                    # the HAM clock gate never sees an idle window and the PE
                    # stays at 2.4 GHz (idle chunks re-throttle it to 1.2).
                    for _f in range(N_FILL):
                        nc.tensor.ldweights(weights=xT[:, _f, 0:128])
                    # previous group's epilogue, spread across this loop
                    if pending is not None:
                        if gi == 1:
                            recT = emit_denT(*pending)
                        elif gi in (3, 5, 7, 9):
                            emit_fp_one(pending[0], (gi - 3) // 2, recT)
                flush_pv(pv_queue)

                oc = o[0:DH, :]
                nc.vector.tensor_copy(out=oN[:, q0 : q0 + NQ], in_=oc)
                den = sb_io.tile([1, NQ], f32, name="den")
                nc.vector.tensor_copy(out=den, in_=o[DH : DH + 1, :])
                pending = (q0, den)

        recT = emit_denT(*pending)
        for t in range(NQ // 128):
            emit_fp_one(pending[0], t, recT)

    nc.compile()
    return nc


def make_in_maps(x, Wq, Wk, Wv, Wo):
    bf16 = ml_dtypes.bfloat16
    # x transposed to [feat, tok] and chunked: [NCH, 128, TOK]
    x_bf = np.ascontiguousarray(
        x.reshape(TOK, D).T.reshape(NCH, 128, TOK)
    ).astype(bf16)
    in_maps = []
    for h in range(H):
        sl = slice(h * DH, (h + 1) * DH)
        in_maps.append(
            {
                "x": x_bf,
                "wq": np.ascontiguousarray(Wq[sl, :].T).astype(bf16),
                "wk": np.ascontiguousarray(Wk[sl, :].T).astype(bf16),
                "wv": np.ascontiguousarray(Wv[sl, :].T).astype(bf16),
                "wo": np.ascontiguousarray(Wo[:, sl].T).astype(bf16),
            }
        )
    return in_maps


def _install_ntff_shim():
    """The axon boot skips registering the NTFF profile hook when the image's
    antenv lacks axon_hooks; register an equivalent shim so trace=True works."""
    import types

    if "antenv.axon_hooks" in sys.modules:
        return
    try:
        from trn_agent_boot.trn_boot import _ntff_profile_via_ctypes

        hook = _ntff_profile_via_ctypes("/opt/axon/libaxon_pjrt.so")
    except Exception:
        hook = None
    mod = types.ModuleType("antenv.axon_hooks")
    mod.get_axon_ntff_profile_hook = lambda: hook
    sys.modules["antenv.axon_hooks"] = mod


def run(x, Wq, Wk, Wv, Wo, bo, trace=False):
    from concourse.bass_utils import run_bass_kernel_spmd

    if trace:
        _install_ntff_shim()

    nc = build_bass()
    in_maps = make_in_maps(x, Wq, Wk, Wv, Wo)
    res = run_bass_kernel_spmd(nc, in_maps, core_ids=list(range(H)), trace=trace)
    acc = np.zeros((TOK, D), dtype=np.float32)
    for r in res.results:
        acc += r["out"]
    acc += np.asarray(bo, dtype=np.float32)[None, :]
    return acc.reshape(B, N, D), res


def kernel(x, Wq, Wk, Wv, Wo, bo):
    out, _ = run(
        np.asarray(x, dtype=np.float32),
        np.asarray(Wq, dtype=np.float32),
        np.asarray(Wk, dtype=np.float32),
        np.asarray(Wv, dtype=np.float32),
        np.asarray(Wo, dtype=np.float32),
        np.asarray(bo, dtype=np.float32),
    )
    return out



# revision 7
# speedup vs baseline: 1.5227x; 1.5227x over previous
"""Trainium2 Bass kernel for nn_CrossAttention_82429012345074.

8-head self-attention, B=2, N=4096, d_model=512, 8 heads x 64 dim.

Sharding: one head per NeuronCore (8 heads / 8 cores) — tensor parallel.
Host sums the 8 per-head partial outputs (each divided by its softmax
denominator, which the device ships separately) and adds the bias.

Per-core device kernel (bf16 matmuls, fp32 accumulation):
  phase A   : per 512-token slab, qk projection as ONE matmul with the
              stacked [Wq;Wk] stationary operand (full 128-col array), and
              v projection into token-partition layout.
  attention : per 512-query group, 11 score chunks of <=3 key blocks.
              Score matmuls are ROW-PAIRED with tile_position — the PE
              runs as 2x (64x128) tiles, two key blocks per 512-cycle
              slot (contraction is only 64).  exp() on ScalarE (the
              critical path: ~262us floor).  PV accumulates unnormalized
              output + denominator row (ones column in v).  Output
              projection (also K=64, row-tiled T0) emits the UNNORMALIZED
              out partial; the denominator is shipped to the host, which
              divides (linearity: (o/den)@Wo == (o@Wo)/den).
  The next batch's projections are interleaved into the attention loop's
  PE idle slots (keeps the PE HAM clock-gate warm with useful work).
"""

import sys

sys.path.insert(0, "/opt/trn_rl_repo")

import numpy as np
import ml_dtypes

B, N, D, H, DH = 2, 4096, 512, 8, 64
TOK = B * N            # 8192
NQ = 512               # query-group width
NCH = D // 128         # 4 feature chunks of x
NJB = N // 128         # 32 key blocks per batch
NTB = TOK // 128       # 64 token blocks
NSLAB = TOK // NQ      # 16 token slabs (8 per batch)
CHUNKS = [3] * 10 + [2]   # key-blocks per exp() chunk (sum = 32)
SCALE = DH ** -0.5
N_FILL = 2             # junk matmuls per score pair (HAM warm-pinning)


def build_bass():
    from contextlib import ExitStack

    import concourse.bass as bass
    import concourse.mybir as mybir
    import concourse.tile as tile
    from concourse import bacc

    f32 = mybir.dt.float32
    bf16 = mybir.dt.bfloat16
    EXP = mybir.ActivationFunctionType.Exp

    nc = bacc.Bacc("TRN2", target_bir_lowering=False, num_devices=8)
    x_d = nc.dram_tensor("x", [NCH, 128, TOK], bf16, kind="ExternalInput")
    wqk_d = nc.dram_tensor("wqk", [D, 128], bf16, kind="ExternalInput")
    wv_d = nc.dram_tensor("wv", [D, DH], bf16, kind="ExternalInput")
    wo_d = nc.dram_tensor("wo", [DH, D], bf16, kind="ExternalInput")
    out_d = nc.dram_tensor("out", [TOK, D], f32, kind="ExternalOutput")
    den_d = nc.dram_tensor("den", [NSLAB, NQ], f32, kind="ExternalOutput")

    with tile.TileContext(nc) as tc, ExitStack() as ctx:
        const = ctx.enter_context(tc.tile_pool(name="const", bufs=1))
        sb_p = ctx.enter_context(tc.tile_pool(name="sb_p", bufs=3))
        sb_io = ctx.enter_context(tc.tile_pool(name="sb_io", bufs=3))
        ps_s = ctx.enter_context(tc.tile_pool(name="ps_s", bufs=2, space="PSUM"))
        ps_o = ctx.enter_context(tc.tile_pool(name="ps_o", bufs=1, space="PSUM"))
        ps_fp = ctx.enter_context(tc.tile_pool(name="ps_fp", bufs=1, space="PSUM"))

        # Long-lived SBUF tensors
        xT = const.tile([128, NCH, TOK], bf16, name="xT")      # x^T, 4 chunks
        qkB = const.tile([128, TOK], bf16, name="qkB")         # [q; k] per token
        qkA = const.tile([128, TOK], bf16, name="qkA")         # [k; q] per token
        vP = const.tile([128, NTB, DH + 1], bf16, name="vP")   # v blocks + ones
        oNd = const.tile([64, TOK], bf16, name="oNd")          # unnorm attn out^T
        wqk = const.tile([128, NCH, 128], bf16, name="wqk")
        wv = const.tile([128, NCH, DH], bf16, name="wv")
        wo = const.tile([64, D], bf16, name="wo")

        nc.vector.memset(vP[:, :, DH : DH + 1], 1.0)

        nc.sync.dma_start(out=wqk, in_=wqk_d[:].rearrange("(c p) m -> p c m", p=128))
        nc.sync.dma_start(out=wv, in_=wv_d[:].rearrange("(c p) d -> p c d", p=128))
        nc.sync.dma_start(out=wo, in_=wo_d[:])

        # Preload the exp activation-table set (hide the ~2.7us load in phase A)
        warm_s = sb_io.tile([1, 1], f32, name="warm_s")
        warm_p = sb_io.tile([1, 1], f32, name="warm_p")
        nc.vector.memset(warm_s, 0.0)
        nc.scalar.activation(out=warm_p, in_=warm_s, func=EXP, scale=1.0)

        # x^T slab DMAs (per 512-token slab, per feature chunk)
        for s in range(NSLAB):
            t0 = s * NQ
            for c in range(NCH):
                nc.sync.dma_start(
                    out=xT[:, c, t0 : t0 + NQ], in_=x_d[c, :, t0 : t0 + NQ]
                )

        def emit_proj_slab(s, qk_psum, vp_psum):
            """Projections for one 512-token slab.

            qk_psum: [128, 512] bank — q rows 0-63, k rows 64-127
            vp_psum: [128, 256] region — 4 token blocks x 64 v-dims
            """
            t0 = s * NQ
            if qk_psum is not None:
                for c in range(NCH):
                    nc.tensor.matmul(
                        qk_psum, lhsT=wqk[:, c, :], rhs=xT[:, c, t0 : t0 + NQ],
                        start=(c == 0), stop=(c == NCH - 1),
                    )
            if vp_psum is not None:
                for t in range(4):
                    tb0 = t0 + t * 128
                    for c in range(NCH):
                        nc.tensor.matmul(
                            vp_psum[:, t * 64 : t * 64 + 64],
                            lhsT=xT[:, c, tb0 : tb0 + 128], rhs=wv[:, c, :],
                            start=(c == 0), stop=(c == NCH - 1),
                        )

        def emit_proj_copies(s, qk_psum, vp_psum):
            t0 = s * NQ
            nc.vector.tensor_copy(out=qkB[:, t0 : t0 + NQ], in_=qk_psum)
            nc.vector.tensor_copy(
                out=vP[:, s * 4 : s * 4 + 4, 0:DH],
                in_=vp_psum.rearrange("p (t d) -> p t d", t=4),
            )
            # build qkA = [k; q] from qkB = [q; k] (partition swap via DMA)
            nc.sync.dma_start(
                out=qkA[0:64, t0 : t0 + NQ], in_=qkB[64:128, t0 : t0 + NQ]
            )
            nc.sync.dma_start(
                out=qkA[64:128, t0 : t0 + NQ], in_=qkB[0:64, t0 : t0 + NQ]
            )

        # ---- Phase A: batch-0 projections (slabs 0-7) ----
        for s in range(B * 4):
            sA = ps_s.tile([128, 3, NQ], f32, tag="s", name="sA")
            emit_proj_slab(s, sA[:, 0, :], sA[:, 1, 0:256])
            emit_proj_copies(s, sA[:, 0, :], sA[:, 1, 0:256])

        # ---- Attention main loop ----
        # Cycle c of each group: (64,128)-mode window [score pairs + out-proj
        # of the previous group + fillers], exp(c) on ScalarE, (128,128)-mode
        # window [PV(c-1) + interleaved next-batch projections].
        CSTART = [0, 3, 6, 9, 12, 15, 18, 21, 24, 27, 30]  # chunk -> first block
        PAIRS_OF_CYCLE = [
            [0, 1], [2], [3, 4], [5], [6, 7], [8],
            [9, 10], [11], [12, 13], [14], [15],
        ]

        pending = None  # (q0,) of the previous group awaiting out-projection

        def emit_fp_one(pq0, t):
            tt0 = pq0 + t * 128
            fp = ps_fp.tile([128, D], f32, tag="fp", name="fp")
            nc.tensor.matmul(
                fp, lhsT=oNd[:, tt0 : tt0 + 128], rhs=wo,
                start=True, stop=True, tile_position=(0, 0),
            )
            ob = sb_io.tile([128, D], f32, tag="ob", name="ob")
            nc.vector.tensor_copy(out=ob, in_=fp)
            nc.sync.dma_start(out=out_d[tt0 : tt0 + 128, :], in_=ob)

        for g in range(2 * NSLAB // 2):  # 16 groups
            b, qg = g // 8, g % 8
            q0 = b * N + qg * NQ
            o = ps_o.tile([128, NQ], f32, tag="o", name="o")
            s_tiles = {}

            def s_slice(blk):
                c, i = blk // 3, blk % 3
                if c not in s_tiles:
                    s_tiles[c] = ps_s.tile([128, 3, NQ], f32, tag="s", name="s")
                return s_tiles[c][:, i, :]

            # b1 projection slab interleaved into groups 0..7 (all of
            # batch 1's q/k/v ready before batch-1 attention starts)
            proj_slab = 8 + g if g < 8 else None

            pv_queue = None  # chunk index awaiting PV emission
            p_tiles = {}

            def flush_pv(c):
                gsz = CHUNKS[c]
                p = p_tiles[c]
                for i in range(gsz):
                    j = CSTART[c] + i
                    nc.tensor.matmul(
                        o[0 : DH + 1, :], lhsT=vP[:, b * NJB + j, :],
                        rhs=p[:, i, :],
                        start=(j == 0), stop=(j == NJB - 1),
                    )

            for c in range(11):
                # ---- (64,128)-mode window: score pairs ----
                for p_i in PAIRS_OF_CYCLE[c]:
                    b0, b1_ = 2 * p_i, 2 * p_i + 1
                    s0 = s_slice(b0)
                    j0 = b * N + b0 * 128
                    # HAM filler: junk matmuls into the slice the real pair
                    # overwrites (start=True clears them)
                    for _f in range(N_FILL):
                        nc.tensor.matmul(
                            s0[:, 0:64], lhsT=qkA[0:64, 0:128],
                            rhs=qkB[0:64, 0:64],
                            start=True, stop=True, tile_position=(0, 0),
                        )
                    nc.tensor.matmul(
                        s0, lhsT=qkA[0:64, j0 : j0 + 128],
                        rhs=qkB[0:64, q0 : q0 + NQ],
                        start=True, stop=True, tile_position=(0, 0),
                    )
                    if b1_ < NJB:
                        s1 = s_slice(b1_)
                        j1 = b * N + b1_ * 128
                        nc.tensor.matmul(
                            s1, lhsT=qkB[64:128, j1 : j1 + 128],
                            rhs=qkA[64:128, q0 : q0 + NQ],
                            start=True, stop=True, tile_position=(64, 0),
                        )
                # proj-psum evacuation copies FIRST (so the fp-slot WAR chain
                # on the DVE queue stays acyclic), then the out-projection of
                # the previous group.
                if proj_slab is not None:
                    if c == 6:
                        nc.vector.tensor_copy(
                            out=qkB[:, proj_slab * NQ : proj_slab * NQ + NQ],
                            in_=qk_ps,
                        )
                    elif c == 8:
                        t0p = proj_slab * NQ
                        nc.vector.tensor_copy(
                            out=vP[:, proj_slab * 4 : proj_slab * 4 + 4, 0:DH],
                            in_=vp_ps.rearrange("p (t d) -> p t d", t=4),
                        )
                        nc.sync.dma_start(
                            out=qkA[0:64, t0p : t0p + NQ],
                            in_=qkB[64:128, t0p : t0p + NQ],
                        )
                        nc.sync.dma_start(
                            out=qkA[64:128, t0p : t0p + NQ],
                            in_=qkB[0:64, t0p : t0p + NQ],
                        )
                if pending is not None and c in (2, 4, 6, 8):
                    emit_fp_one(pending, (c - 2) // 2)

                # ---- exp(c) on ScalarE ----
                gsz = CHUNKS[c]
                pt = sb_p.tile([128, 3, NQ], bf16, tag="p", name="pt")
                nc.scalar.activation(
                    out=pt[:, 0:gsz, :], in_=s_tiles[c][:, 0:gsz, :],
                    func=EXP, scale=SCALE,
                )
                p_tiles[c] = pt

                # ---- (128,128)-mode window: deferred PV + projections ----
                if pv_queue is not None:
                    flush_pv(pv_queue)
                pv_queue = c
                if proj_slab is not None:
                    if c == 4:
                        qk_ps = ps_fp.tile([128, NQ], f32, tag="fp", name="qk_ps")
                        emit_proj_slab(proj_slab, qk_ps, None)
                    elif c == 6:
                        vp_ps = ps_fp.tile([128, 256], f32, tag="fp", name="vp_ps")
                        emit_proj_slab(proj_slab, None, vp_ps)

            flush_pv(10)

            # group epilogue: unnormalized attn out (bf16) + denominator (f32)
            nc.vector.tensor_copy(out=oNd[:, q0 : q0 + NQ], in_=o[0:DH, :])
            denb = sb_io.tile([128, NQ], f32, tag="den", name="denb")
            nc.vector.tensor_copy(out=denb[DH : DH + 1, :], in_=o[DH : DH + 1, :])
            nc.sync.dma_start(
                out=den_d[g, :].unsqueeze(0), in_=denb[DH : DH + 1, :]
            )
            pending = q0

        for t in range(4):
            emit_fp_one(pending, t)

    nc.compile()
    return nc


def make_in_maps(x, Wq, Wk, Wv, Wo):
    bf16 = ml_dtypes.bfloat16
    x_bf = np.ascontiguousarray(
        x.reshape(TOK, D).T.reshape(NCH, 128, TOK)
    ).astype(bf16)
    in_maps = []
    for h in range(H):
        sl = slice(h * DH, (h + 1) * DH)
        wqk = np.concatenate([Wq[sl, :].T, Wk[sl, :].T], axis=1)  # [512, 128]
        in_maps.append(
            {
                "x": x_bf,
                "wqk": np.ascontiguousarray(wqk).astype(bf16),
                "wv": np.ascontiguousarray(Wv[sl, :].T).astype(bf16),
                "wo": np.ascontiguousarray(Wo[:, sl].T).astype(bf16),
            }
        )
    return in_maps


def _install_ntff_shim():
    """The axon boot skips registering the NTFF profile hook when the image's
    antenv lacks axon_hooks; register an equivalent shim so trace=True works."""
    import types

    if "antenv.axon_hooks" in sys.modules:
        return
    try:
        from trn_agent_boot.trn_boot import _ntff_profile_via_ctypes

        hook = _ntff_profile_via_ctypes("/opt/axon/libaxon_pjrt.so")
    except Exception:
        hook = None
    mod = types.ModuleType("antenv.axon_hooks")
    mod.get_axon_ntff_profile_hook = lambda: hook
    sys.modules["antenv.axon_hooks"] = mod


def run(x, Wq, Wk, Wv, Wo, bo, trace=False):
    from concourse.bass_utils import run_bass_kernel_spmd

    if trace:
        _install_ntff_shim()

    nc = build_bass()
    in_maps = make_in_maps(x, Wq, Wk, Wv, Wo)
    res = run_bass_kernel_spmd(nc, in_maps, core_ids=list(range(H)), trace=trace)
    acc = np.zeros((TOK, D), dtype=np.float32)
    for r in res.results:
        den = r["den"].reshape(TOK, 1)
        acc += r["out"] / den
    acc += np.asarray(bo, dtype=np.float32)[None, :]
    return acc.reshape(B, N, D), res


def kernel(x, Wq, Wk, Wv, Wo, bo):
    out, _ = run(
        np.asarray(x, dtype=np.float32),
        np.asarray(Wq, dtype=np.float32),
        np.asarray(Wk, dtype=np.float32),
        np.asarray(Wv, dtype=np.float32),
        np.asarray(Wo, dtype=np.float32),
        np.asarray(bo, dtype=np.float32),
    )
    return out
